# revision 57
# baseline (speedup 1.0000x reference)
"""Per-neuron grouped MLP (conv-style) kernel for Trainium2, 8 NeuronCores.

Math (per group d):  h = x[:, d, :] @ W1[d].T + b1[d]; g = gelu(h); out[:, d] = g @ W2[d] + b2[d]
  x: [B=512, D=2048, M=128], W1: [D, H=128, M], b1: [D, H], W2: [D, H], b2: [D]

Strategy (v4, ~142 us cost-model makespan per core; ScalarE gelu-bound):
  - Shard on D: each of 8 cores owns D_LOC = 256 independent per-neuron MLPs.
  - Host pre-packs x as [chunk, m, d, b] so each 16-d chunk DMA is one dense
    contiguous [128, 16*512] fp16 transfer (16 KB per partition line). All
    input DMAs share the SP HWDGE FIFO so issue order == completion order;
    one 16-d W1 slice is interleaved ahead of each x chunk, and chunk 0 is
    split 8+8 so the first matmul starts after ~1.6 MB.
  - W1 ends up fully SBUF-resident ([m, d, h] = 64 KB/partition).
  - Per d: MM1 psum1[H, B] = W1[d].T @ x[d]  (K=M=128, N=512).
    Gelu per TRIPLE of d's: one ScalarE activation, FD=1536 (3 PSUM banks),
    reading PSUM directly. psum1 pool = 2x3 banks, psum2 pool = 2x1 banks
    (all 8 banks). ScalarE is the wall: 16.7M gelu elems/core at 1
    elem/lane/cycle ~= 109 us + per-instruction overhead.
  - MM2 packs 4 d's into one PSUM bank via col-tiled matmuls
    (tile_position=(0,32j) -> rows {0,32,64,96}), one DVE copy per quad to
    SBUF, and one strided DMA per 16 d writes outT[d, b] fp32.
  - The PE stream is software-pipelined one 12-d group deep (quads of group
    g are emitted after the MM1 triples of group g+1) so the in-order PE
    queue never waits on the gelu it just enabled.
  - b1 is zero in this problem; a bias-mode fallback applies it per-d.
    b2 is added on the host (it is outside the nonlinearity).
  - An int8 mode (x quantized per-(d,m), scales folded into W1, SWDGE cast
    DMA) validates at rel err 6.8e-3 but is disabled: the cast path gains
    nothing in the DMA cost model and adds SWDGE queue overhead.
"""

import numpy as np

B, D, M, H = 512, 2048, 128, 128
N_CORES = 8
D_LOC = D // N_CORES  # 256
CHUNK = 16           # d's per x DMA chunk
NCHUNK = D_LOC // CHUNK
QUAD = 4             # d's per MM2 packing group
TRIPLE = 3           # d's per psum1/ACT group (FD=1536 activations)
GGRP = 12            # d's per g tile (aligns gelu triples and MM2 quads)
W1_SLICES = 16       # W1 preload DMA count
OFF_GROUPS = 0       # offload t0 of every OFF_GROUPS-th g-group (0=off)
OFF_MAXGRP = 18      # no offloads near the tail

OCH = 16             # d's per output-store tile

PRECISION = "fp16"   # "fp16" | "int8"
INT8_BOOST = 128.0   # power-of-2 fold boost for int8 mode
# gelu(x) ~= x*(0.5 + x*(C1 + s*(C3 + s*C5))), s = x^2, fit on |x|<=1.7
GELU_C1, GELU_C3, GELU_C5 = 0.39874054, -0.06500604, 0.00743408

_NC_CACHE = {}


def build_nc(bias_mode: bool, prec: str = PRECISION, reps: int = 1):
    key = (bias_mode, prec, reps)
    if key in _NC_CACHE:
        return _NC_CACHE[key]

    import concourse.bacc as bacc
    import concourse.mybir as mybir
    import concourse.tile as tile

    f32 = mybir.dt.float32
    dt = mybir.dt.float16
    x_dram_dt = mybir.dt.int8 if prec == "int8" else dt

    nc = bacc.Bacc("TRN2", target_bir_lowering=False, debug=False, num_devices=N_CORES)
    xT = nc.dram_tensor("xT", [NCHUNK, M, CHUNK, B], x_dram_dt, kind="ExternalInput").ap()
    w1T = nc.dram_tensor("w1T", [M, D_LOC, H], dt, kind="ExternalInput").ap()
    w2T = nc.dram_tensor("w2T", [H, D_LOC], dt, kind="ExternalInput").ap()
    b1T = nc.dram_tensor("b1T", [H, D_LOC], f32, kind="ExternalInput").ap()
    outT = nc.dram_tensor("outT", [D_LOC, B], f32, kind="ExternalOutput").ap()

    with (
        tile.TileContext(nc) as tc,
        tc.tile_pool(name="singles", bufs=1) as singles,
        tc.tile_pool(name="xp", bufs=3) as xp,
        tc.tile_pool(name="gp", bufs=5) as gp,
        tc.tile_pool(name="dvp", bufs=max(1, 2 * bool(OFF_GROUPS))) as dvp,
        tc.tile_pool(name="op", bufs=2) as op_pool,
        tc.tile_pool(name="ps1", bufs=2, space="PSUM") as ps1,
        tc.tile_pool(name="ps2", bufs=2, space="PSUM") as ps2,
    ):
        w2_sb = singles.tile([H, D_LOC], dt)
        nc.scalar.dma_start(out=w2_sb[:], in_=w2T[:])
        b1_sb = None
        if bias_mode:
            b1_sb = singles.tile([H, D_LOC], f32)
            nc.sync.dma_start(out=b1_sb[:], in_=b1T[:])
        # W1 resident: [m, d, h]. All input DMAs share the SP HWDGE FIFO so
        # issue order = completion order; W1 slices are interleaved between
        # early x chunks by _body (slice s gates chunks 2s..2s+1).
        w1_sb = singles.tile([M, D_LOC, H], dt)

        for _rep in range(reps):
            _body(nc, tc, bias_mode, prec, dt, f32,
                  xT, w1T, outT, w1_sb, w2_sb, b1_sb,
                  xp, gp, dvp, op_pool, ps1, ps2)

    nc.compile()
    _NC_CACHE[key] = nc
    return nc


def _body(nc, tc, bias_mode, prec, dt, f32, xT, w1T, outT, w1_sb, w2_sb, b1_sb,
          xp, gp, dvp, op_pool, ps1, ps2):
    import concourse.mybir as mybir

    GELU = mybir.ActivationFunctionType.Gelu
    ALU = mybir.AluOpType
    act_scale = (1.0 / INT8_BOOST) if prec == "int8" else 1.0
    inv_boost = act_scale  # DVE chain folds the same pre-scale into step 1
    NQ = CHUNK // QUAD
    DS = D_LOC // W1_SLICES

    def x_dma(out, in_):
        if prec == "int8":
            nc.gpsimd.dma_start(out=out, in_=in_)  # SWDGE casts i8->f16
        else:
            nc.sync.dma_start(out=out, in_=in_)

    def w1_dma(s):
        nc.sync.dma_start(
            out=w1_sb[:, s * DS : (s + 1) * DS, :],
            in_=w1T[:, s * DS : (s + 1) * DS, :],
        )

    x_tiles = {}

    DPS = D_LOC // W1_SLICES  # d's per W1 slice

    def get_x(c):
        if c not in x_tiles:
            for s in range(c * CHUNK // DPS, ((c + 1) * CHUNK - 1) // DPS + 1):
                w1_dma(s)  # W1 slice ahead of the chunk it gates
            x_sb = xp.tile([M, CHUNK, B], dt, name="x_sb")
            if c == 0:
                for lo, hi in ((0, 8), (8, CHUNK)):
                    x_dma(x_sb[:, lo:hi, :], xT[c][:, lo:hi, :])
            else:
                x_dma(x_sb[:], xT[c])
            x_tiles[c] = x_sb
        return x_tiles[c]

    def dve_gelu(g_dst, p1, n):
        """g_dst = gelu(p1 * inv_boost) via a 6-op DVE polynomial chain.

        cs = p1 * inv_boost (psum fp32 -> sbuf fp16); s = cs^2;
        u1 = C5*s + C3; u2 = u1*s; u1 = (u2 + C1)*cs; g = (u1 + 0.5)*cs.
        """
        cs = dvp.tile([H, n * B], dt, name="cs")
        sq = dvp.tile([H, n * B], dt, name="sq")
        u1 = dvp.tile([H, n * B], dt, name="u1")
        u2 = dvp.tile([H, n * B], dt, name="u2")
        # psum -> sbuf (+ int8 descale) on DVE; polynomial on the idle GPSIMD
        nc.vector.tensor_scalar(cs[:], p1[:], inv_boost, None, ALU.mult)
        nc.gpsimd.tensor_tensor(sq[:], cs[:], cs[:], ALU.mult)
        nc.gpsimd.tensor_scalar(u1[:], sq[:], GELU_C5, GELU_C3, ALU.mult, ALU.add)
        nc.gpsimd.scalar_tensor_tensor(u2[:], u1[:], 0.0, sq[:], ALU.add, ALU.mult)
        nc.gpsimd.scalar_tensor_tensor(u1[:], u2[:], GELU_C1, cs[:], ALU.add, ALU.mult)
        nc.gpsimd.scalar_tensor_tensor(g_dst, u1[:], 0.5, cs[:], ALU.add, ALU.mult)

    def mm1_group(d0, n, g_sb, goff, offload):
        """n d's starting at d0: MM1 into one psum tile, gelu into g_sb."""
        p1 = ps1.tile([H, n * B], f32, name="p1")
        for j in range(n):
            dd = d0 + j
            nc.tensor.matmul(
                p1[:, j * B : (j + 1) * B],
                lhsT=w1_sb[:, dd, :],
                rhs=get_x(dd // CHUNK)[:, dd % CHUNK, :],
                start=True,
                stop=True,
            )
        if bias_mode:
            for j in range(n):
                dd = d0 + j
                nc.scalar.activation(
                    g_sb[:, goff + j * B : goff + (j + 1) * B],
                    p1[:, j * B : (j + 1) * B],
                    GELU,
                    bias=b1_sb[:, dd : dd + 1],
                    scale=act_scale,
                )
        elif offload:
            dve_gelu(g_sb[:, goff : goff + n * B], p1, n)
        else:
            nc.scalar.activation(
                g_sb[:, goff : goff + n * B], p1[:], GELU, scale=act_scale
            )
        return p1

    def mm2_quad(d0, g_sb, goff, o_sb, oq):
        p2 = ps2.tile([128, B], f32, name="p2")
        for j in range(QUAD):
            dd = d0 + j
            nc.tensor.matmul(
                p2[32 * j : 32 * j + 1, :],
                lhsT=w2_sb[:, dd : dd + 1],
                rhs=g_sb[:, goff + j * B : goff + (j + 1) * B],
                start=True,
                stop=True,
                tile_position=(0, 32 * j),
            )
        nc.vector.tensor_copy(o_sb[:, oq, :], p2[:])

    # initialize both psum2 slots once so the full-tile DVE copy never reads
    # uninitialized PSUM (rows other than {0,32,64,96} are copied but unused)
    for _ in range(2):
        pz = ps2.tile([128, B], f32, name="p2")
        nc.vector.memset(pz[:], 0.0)

    o_tiles = {}

    def quads(d0, n_grp, g_sb):
        """MM2 quads + psum2 copy + output store for one g group."""
        for q0 in range(0, n_grp, QUAD):
            dd = d0 + q0
            c = dd // OCH
            if c not in o_tiles:
                o_tiles[c] = op_pool.tile([128, OCH // QUAD, B], f32, name="o_sb")
            mm2_quad(dd, g_sb, q0 * B, o_tiles[c], (dd % OCH) // QUAD)
            half = OCH // 2
            if dd == D_LOC - QUAD and dd % OCH == OCH - QUAD:
                # split the final store so the tail drains sooner; by now the
                # scalar HWDGE ring is idle and has lower first-byte latency
                for h0 in (0, half):
                    nc.scalar.dma_start(
                        out=outT[c * OCH + h0 : c * OCH + h0 + half].rearrange(
                            "(q j) b -> j q b", j=QUAD
                        ),
                        in_=o_tiles[c][0::32, h0 // QUAD : (h0 + half) // QUAD, :],
                    )
            elif dd % OCH == OCH - QUAD:  # last quad of the store group
                nc.gpsimd.dma_start(
                    out=outT[c * OCH : (c + 1) * OCH].rearrange(
                        "(q j) b -> j q b", j=QUAD
                    ),
                    in_=o_tiles[c][0::32, :, :],
                )

    # software pipeline: emit quads(g) one group late so the in-order PE
    # stream never waits on the gelu it just enabled
    d0 = 0
    pending = None  # (d0, n_grp, g_sb) awaiting MM2
    while d0 < D_LOC:
        n_grp = min(GGRP, D_LOC - d0)  # 12 or the 4-d tail
        g_sb = gp.tile([H, n_grp * B], dt, name="g_sb")
        gs = TRIPLE if n_grp % TRIPLE == 0 else 2
        for t0 in range(0, n_grp, gs):
            mm1_group(d0 + t0, min(gs, n_grp - t0), g_sb, t0 * B, False)
        if pending is not None:
            quads(*pending)
        pending = (d0, n_grp, g_sb)
        d0 += n_grp
    quads(*pending)


def prepare_in_maps(x, W1, b1, W2, prec: str = PRECISION):
    """Host-side shard + pack. Returns list of 8 per-core input dicts."""
    x = np.asarray(x, dtype=np.float32)
    W1 = np.asarray(W1, dtype=np.float32)
    b1 = np.asarray(b1, dtype=np.float32)
    W2 = np.asarray(W2, dtype=np.float32)

    in_maps = []
    for k in range(N_CORES):
        sl = slice(k * D_LOC, (k + 1) * D_LOC)
        xk = x[:, sl, :]          # [B, D_LOC, M]
        w1k = W1[sl]              # [D_LOC, H, M]
        if prec == "int8":
            # per-(d,m) symmetric int8 over b; scales folded into W1
            s = np.max(np.abs(xk), axis=0) / 127.0  # [D_LOC, M]
            s = np.maximum(s, 1e-12)
            xq = np.clip(np.round(xk / s[None]), -127, 127).astype(np.int8)
            xT_k = np.ascontiguousarray(
                xq.transpose(2, 1, 0)  # [M, D_LOC, B]
                .reshape(M, NCHUNK, CHUNK, B)
                .transpose(1, 0, 2, 3)  # [NCHUNK, M, CHUNK, B]
            )
            w1f = w1k * (s[:, None, :] * INT8_BOOST)  # fold scales into W1
        else:
            xT_k = np.ascontiguousarray(
                xk.transpose(2, 1, 0)
                .reshape(M, NCHUNK, CHUNK, B)
                .transpose(1, 0, 2, 3),
                dtype=np.float16,
            )
            w1f = w1k
        # w1T layout [m, d, h]
        w1T_k = np.ascontiguousarray(w1f.transpose(2, 0, 1), dtype=np.float16)
        w2T_k = np.ascontiguousarray(W2[sl].T, dtype=np.float16)
        b1T_k = np.ascontiguousarray(b1[sl].T, dtype=np.float32)
        in_maps.append({"xT": xT_k, "w1T": w1T_k, "w2T": w2T_k, "b1T": b1T_k})
    return in_maps


def assemble_output(results, b2):
    outT_full = np.concatenate([r["outT"] for r in results], axis=0)  # [D, B]
    out = outT_full.T  # [B, D]
    b2 = np.asarray(b2, dtype=np.float32)
    if np.any(b2):
        out = out + b2[None, :]
    return np.ascontiguousarray(out)


def kernel(pre_activation_history, W1, b1, W2, b2):
    from concourse.bass_utils import run_bass_kernel_spmd

    b1 = np.asarray(b1, dtype=np.float32)
    bias_mode = bool(np.any(b1))
    nc = build_nc(bias_mode)
    in_maps = prepare_in_maps(pre_activation_history, W1, b1, W2)
    res = run_bass_kernel_spmd(nc, in_maps, core_ids=list(range(N_CORES)))
    return assemble_output(res.results, b2)
